# revision 8
# baseline (speedup 1.0000x reference)
"""KuramotoCell Bass kernel for 8 TRN2 NeuronCores.

Math: coupling[b,i] = sum_j Wh[i,j] * sin(s[b,i] - s[b,j])
                    = sin(s_bi) * (Wh @ cos(s_b))_i - cos(s_bi) * (Wh @ sin(s_b))_i
so the O(B*n^2) pairwise term is two [B,n]x[n,n] matmuls. Memory roofline is one
pass over Wh (16.8 MB). Sharding: rows of Wh (the output i-axis) across the 8
cores, 256 rows each -- every term of the output block is local, no collectives.

Per core (i0 = 256*core):
  lhsT trig[128(j), 64]   = [cos'(s_j) | sin'(s_j)] per j-tile (stationary)
  rhs  whT  [128(j), 256] = Wh[i0:i0+256, jtile].T  (moving, fp32r)
  psum[64, 256] accumulates M'[b,i] (rows 0:32) and S'[b,i] (rows 32:64)
where cos'(u) = cos(u - pi) = -cos(u), sin'(u) = sin(u - pi) = -sin(u): the Sin
activation table is only accurate on ~(-3.4, 3.4), so angles are shifted by -pi
into [-pi, pi); the sign flips cancel in  coupling = sin'*M' - cos'*S'.

x @ Wi_w.T + (Wi_b + omega) + state rides on a second small matmul: xaug is
[x.T; 1; I_32] (K=61) against [Wi_w_blk.T; Wi_b+omega; state_blk], so the bias
and the +state term cost nothing extra.

Combine uses one [64,256] DVE mul against the stacked psum: combo rows 0:32 =
sin'(s_i), rows 32:64 = -(-cos'(s_i)) ... = Sin(|s_i - pi| - pi/2) = -cos'(s_i),
so coupling = prod[0:32] + prod[32:64] in a single partition-offset add.

mod 2pi via floor by magic-number rounding: with t = acc/2pi + (OFF - 0.5 + MAGIC),
k = t - MAGIC = floor(acc/2pi + OFF), r = acc - 2pi*k + 2pi*OFF  in [0, 2pi).
"""
import sys

for _p in ("/opt/trn_rl_repo", "/root/.axon_site/_ro/trn_rl_repo"):
    if _p not in sys.path:
        sys.path.insert(0, _p)

import numpy as np
import concourse.mybir as mybir
import concourse.tile as tile
from concourse import bacc
from concourse.bass_utils import run_bass_kernel_spmd

F32 = mybir.dt.float32
F32R = mybir.dt.float32r
AF = mybir.ActivationFunctionType
OP = mybir.AluOpType

TWO_PI = float(2.0 * np.pi)
PI = float(np.pi)
HALF_PI = float(np.pi / 2)
INV_2PI = float(1.0 / (2.0 * np.pi))
MAGIC = 12582912.0  # 1.5 * 2**23: adding then subtracting forces RNE to integer
OFF = 2.0           # shift so acc/2pi + OFF - 0.5 > 0 => rne(x-0.5) = floor(x)

B = 32          # batch
NH = 2048       # n_hid
NI = 28         # n_inp
NCORES = 8
IBLK = NH // NCORES       # 256 output rows per core
JT = NH // 128            # 16 contraction tiles
NCHUNK = 4                # whT DMA chunks (4 j-tiles each)
PER = JT // NCHUNK
KAUG = NI + 1 + B         # x rows + ones row + identity rows


def _build():
    nc = bacc.Bacc("TRN2", target_bir_lowering=False, debug=False,
                   num_devices=NCORES)
    whT_d = nc.dram_tensor("whT", [NCHUNK, 128, PER * IBLK], F32R,
                           kind="ExternalInput")
    stt_d = nc.dram_tensor("stt", [128, JT * B], F32, kind="ExternalInput")
    stblk_d = nc.dram_tensor("stblk", [B, IBLK], F32, kind="ExternalInput")
    xaug_d = nc.dram_tensor("xaug", [KAUG, B], F32, kind="ExternalInput")
    wiaug_d = nc.dram_tensor("wiaug", [KAUG, IBLK], F32, kind="ExternalInput")
    out_d = nc.dram_tensor("out", [B, IBLK], F32, kind="ExternalOutput")

    with tile.TileContext(nc) as tc:
        with (
            tc.tile_pool(name="sb", bufs=1) as sb,
            tc.tile_pool(name="ps", bufs=1, space="PSUM") as ps,
        ):
            neg_pi = sb.tile([128, 1], F32)
            nc.vector.memset(neg_pi[:, :], -PI)
            half_pi = sb.tile([128, 1], F32)
            nc.vector.memset(half_pi[:, :], HALF_PI)
            neg_half_pi = sb.tile([128, 1], F32)
            nc.vector.memset(neg_half_pi[:, :], -HALF_PI)
            # dummy Sin: pulls the ACT table load off the critical path
            warm = sb.tile([128, 1], F32)
            nc.scalar.activation(warm[:, :], neg_pi[:, :], AF.Sin,
                                 bias=half_pi[:, 0:1])

            # small inputs first, then state, then the Wh stream
            stblk = sb.tile([B, IBLK], F32)
            nc.sync.dma_start(stblk[:, :], stblk_d[:, :])
            xaug = sb.tile([KAUG, B], F32)
            nc.sync.dma_start(xaug[:, :], xaug_d[:, :])
            wiaug = sb.tile([KAUG, IBLK], F32)
            nc.sync.dma_start(wiaug[:, :], wiaug_d[:, :])
            stt = sb.tile([128, JT * B], F32)
            nc.sync.dma_start(stt[:, :], stt_d[:, :])
            whc = []
            for c in range(NCHUNK):
                w = sb.tile([128, PER * IBLK], F32R, tag=f"wh{c}")
                nc.sync.dma_start(w[:, :], whT_d[c, :, :])
                whc.append(w)

            # input-projection matmul early: first PE work, tiny operands
            ps_inp = ps.tile([B, IBLK], F32)
            nc.tensor.matmul(ps_inp[:, :], xaug[:, :], wiaug[:, :],
                             start=True, stop=True)

            # i-block trig: srb = sin'(s_i) = Sin(s_i - pi),
            # crbn = -cos'(s_i) = Sin(|s_i - pi| - pi/2)
            srb = sb.tile([B, IBLK], F32)
            babs = sb.tile([B, IBLK], F32)
            crbn = sb.tile([B, IBLK], F32)
            nc.scalar.activation(srb[:, :], stblk[:, :], AF.Sin,
                                 bias=neg_pi[0:B, 0:1])
            nc.scalar.activation(babs[:, :], stblk[:, :], AF.Abs,
                                 bias=neg_pi[0:B, 0:1])
            nc.scalar.activation(crbn[:, :], babs[:, :], AF.Sin,
                                 bias=neg_half_pi[0:B, 0:1])

            # contraction trig + matmuls, pipelined per wh chunk
            trig = sb.tile([128, JT * 64], F32R)
            trig_v = trig[:, :].rearrange("p (t c) -> p t c", c=64)
            stt_v = stt[:, :].rearrange("p (t c) -> p t c", c=B)
            tabs = sb.tile([128, JT * B], F32)
            tabs_v = tabs[:, :].rearrange("p (t c) -> p t c", c=B)
            ps_ms = ps.tile([64, IBLK], F32)
            for c in range(NCHUNK):
                tv = trig_v[:, c * PER:(c + 1) * PER, :]
                sv = stt_v[:, c * PER:(c + 1) * PER, :]
                av = tabs_v[:, c * PER:(c + 1) * PER, :]
                nc.scalar.activation(tv[:, :, B:64], sv[:, :, :], AF.Sin,
                                     bias=neg_pi[:, 0:1])
                nc.scalar.activation(av[:, :, :], sv[:, :, :], AF.Abs,
                                     bias=neg_pi[:, 0:1])
                nc.scalar.activation(tv[:, :, 0:B], av[:, :, :], AF.Sin,
                                     bias=half_pi[:, 0:1], scale=-1.0)
                for q in range(PER):
                    t = c * PER + q
                    nc.tensor.matmul(
                        ps_ms[:, :],
                        trig[:, 64 * t: 64 * t + 64],
                        whc[c][:, IBLK * q: IBLK * (q + 1)],
                        start=(t == 0),
                        stop=(t == JT - 1),
                    )

            # combine: coupling = srb*M' + crbn*S'; acc += inp(+bias+omega+state)
            t1 = sb.tile([B, IBLK], F32)
            t2 = sb.tile([B, IBLK], F32)
            nc.vector.tensor_mul(t1[:, :], srb[:, :], ps_ms[0:B, :])
            nc.vector.tensor_mul(t2[:, :], crbn[:, :], ps_ms[B:64, :])
            acc = sb.tile([B, IBLK], F32)
            nc.vector.tensor_add(acc[:, :], t1[:, :], t2[:, :])
            nc.vector.tensor_add(acc[:, :], acc[:, :], ps_inp[:, :])

            # mod 2pi: r = acc - 2pi*rne(acc/2pi); r += 2pi*(r<0)
            k = sb.tile([B, IBLK], F32)
            nc.vector.tensor_scalar(k[:, :], acc[:, :], INV_2PI, MAGIC,
                                    OP.mult, OP.add)
            nc.vector.tensor_scalar(k[:, :], k[:, :], -MAGIC, -TWO_PI,
                                    OP.add, OP.mult)
            r = sb.tile([B, IBLK], F32)
            nc.vector.tensor_add(r[:, :], acc[:, :], k[:, :])
            fix = sb.tile([B, IBLK], F32)
            nc.vector.tensor_scalar(fix[:, :], r[:, :], 0.0, TWO_PI,
                                    OP.is_lt, OP.mult)
            nc.vector.tensor_add(r[:, :], r[:, :], fix[:, :])

            nc.sync.dma_start(out_d[:, :], r[:, :])

    nc.compile()
    return nc


_NC_CACHE = None


def _get_nc():
    global _NC_CACHE
    if _NC_CACHE is None:
        _NC_CACHE = _build()
    return _NC_CACHE


def make_in_maps(x, state, Wi_w, Wi_b, Wh, omega):
    x = np.ascontiguousarray(x, dtype=np.float32)
    state = np.ascontiguousarray(state, dtype=np.float32)
    Wi_w = np.ascontiguousarray(Wi_w, dtype=np.float32)
    Wi_b = np.ascontiguousarray(Wi_b, dtype=np.float32)
    Wh = np.ascontiguousarray(Wh, dtype=np.float32)
    omega = np.ascontiguousarray(omega, dtype=np.float32)

    # [2048, 32] -> 16 tiles of [128, 32] laid side by side: [128, 16*32]
    stt = np.ascontiguousarray(
        state.T.reshape(JT, 128, B).transpose(1, 0, 2).reshape(128, JT * B))
    bias_full = Wi_b + omega

    in_maps = []
    for c in range(NCORES):
        i0 = c * IBLK
        blk = Wh[i0:i0 + IBLK, :].T            # [2048, 256]
        whT = np.ascontiguousarray(
            blk.reshape(JT, 128, IBLK).transpose(1, 0, 2).reshape(128, JT * IBLK))
        whT = np.ascontiguousarray(
            whT.reshape(128, NCHUNK, PER * IBLK).transpose(1, 0, 2))
        xaug = np.zeros((KAUG, B), dtype=np.float32)
        xaug[:NI] = x.T
        xaug[NI] = 1.0
        xaug[NI + 1:] = np.eye(B, dtype=np.float32)
        wiaug = np.empty((KAUG, IBLK), dtype=np.float32)
        wiaug[:NI] = Wi_w[i0:i0 + IBLK, :].T
        wiaug[NI] = bias_full[i0:i0 + IBLK]
        wiaug[NI + 1:] = state[:, i0:i0 + IBLK]
        in_maps.append({
            "whT": whT,
            "stt": stt,
            "stblk": np.ascontiguousarray(state[:, i0:i0 + IBLK]),
            "xaug": xaug,
            "wiaug": np.ascontiguousarray(wiaug),
        })
    return in_maps


def kernel(x, state, Wi_w, Wi_b, Wh, omega, _trace=False):
    nc = _get_nc()
    in_maps = make_in_maps(x, state, Wi_w, Wi_b, Wh, omega)
    res = run_bass_kernel_spmd(nc, in_maps, list(range(NCORES)), trace=_trace)
    out = np.concatenate([res.results[c]["out"] for c in range(NCORES)], axis=1)
    if _trace:
        kernel.last_result = res
    return out.astype(np.float32, copy=False)
